# revision 2
# baseline (speedup 1.0000x reference)
"""AttentionAggregation (segment softmax pooling) on 8 Trainium2 cores.

Math (per reference):
    s = tanh(x @ W1 + b1) @ W2 + b2            # [N]
    alpha = segment_softmax(s, batch_idx)      # [N, 1]
    out = segment_sum(x * alpha, batch_idx)    # [4096, 128]
Returns (out, alpha).

Strategy:
  - batch_idx is sorted; 4096 segments -> 512 per core, each segment padded to
    SMAX=640 rows (max actual ~578).  Host lays x out per core in two swizzled
    bf16 copies (natural chunks for pooling matmuls, transposed chunks for the
    score matmuls) so that the device program is fully static and identical
    across cores; all data-dependence lives in the tensors (padding + mask).
  - Scores use exp(s) without max-subtraction (scores are in [-1.2, 1.2] for
    this MLP scale, so no overflow; identical softmax up to fp rounding).
  - Segment sums via a ones-matmul (partition reduction) + free-dim reduce.
  - Pooling accumulates out^T columns in one PSUM bank across all segments.
"""

import numpy as np
import ml_dtypes

N = 2_000_000
D = 128
NSEG = 4096
NCORES = 8
SPC = NSEG // NCORES          # segments per core = 512
SMAX = 640                    # padded rows per segment (5 chunks of 128)
CPS = SMAX // 128             # chunks per segment = 5
GSEGS = 8                     # segments per group
GROUPS = SPC // GSEGS         # 64
GCH = GSEGS * CPS             # chunks per group = 40
GW = GCH * 128                # free-dim width per group = 5120
TOT_CH = SPC * CPS            # 2560 chunks per core
RPAD = SPC * SMAX             # 327680 padded rows per core

BF16 = ml_dtypes.bfloat16

_CACHE = {}


def _build_nc():
    import concourse.bacc as bacc
    import concourse.mybir as mybir
    from concourse.tile import TileContext

    f32 = mybir.dt.float32
    bf16 = mybir.dt.bfloat16

    nc = bacc.Bacc(None, target_bir_lowering=False)

    xn = nc.dram_tensor("xn", [128, RPAD], bf16, kind="ExternalInput")
    xt = nc.dram_tensor("xt", [128, RPAD], bf16, kind="ExternalInput")
    maskb = nc.dram_tensor("maskb", [128, TOT_CH], f32, kind="ExternalInput")
    w1 = nc.dram_tensor("w1", [128, 128], bf16, kind="ExternalInput")
    w2 = nc.dram_tensor("w2", [128, 1], bf16, kind="ExternalInput")
    b1d = nc.dram_tensor("b1d", [128, 1], f32, kind="ExternalInput")
    outp = nc.dram_tensor("outp", [128, SPC], f32, kind="ExternalOutput")
    alph = nc.dram_tensor("alph", [128, TOT_CH], f32, kind="ExternalOutput")

    TANH = mybir.ActivationFunctionType.Tanh
    EXP = mybir.ActivationFunctionType.Exp
    ADD = mybir.AluOpType.add
    AXX = mybir.AxisListType.X

    with TileContext(nc) as tc:
        with (
            tc.tile_pool(name="const", bufs=1) as pc,
            tc.tile_pool(name="xn_p", bufs=3) as px_n,
            tc.tile_pool(name="xt_p", bufs=3) as px_t,
            tc.tile_pool(name="tan_p", bufs=3) as ptan,
            tc.tile_pool(name="soft_p", bufs=3) as pe,
            tc.tile_pool(name="ps_h", bufs=2, space="PSUM") as pp_h,
            tc.tile_pool(name="ps_s", bufs=2, space="PSUM") as pp_s,
            tc.tile_pool(name="ps_c", bufs=1, space="PSUM") as pp_c,
            tc.tile_pool(name="ps_acc", bufs=1, space="PSUM") as pp_acc,
        ):
            w1_sb = pc.tile([128, 128], bf16)
            nc.sync.dma_start(w1_sb[:], w1[:, :])
            w2_sb = pc.tile([128, 1], bf16)
            nc.sync.dma_start(w2_sb[:], w2[:, :])
            b1_sb = pc.tile([128, 1], f32)
            nc.sync.dma_start(b1_sb[:], b1d[:, :])
            mask_sb = pc.tile([128, TOT_CH], f32)
            nc.sync.dma_start(mask_sb[:], maskb[:, :])
            ones_sb = pc.tile([128, 128], f32)
            nc.vector.memset(ones_sb[:], 1.0)
            alpha_sb = pc.tile([128, TOT_CH], f32)
            out_sb = pc.tile([128, SPC], f32)

            pool_acc = pp_acc.tile([128, SPC], f32)

            def emit_pool(xn_g, alphab, g):
                for ci in range(GCH):
                    si, t = divmod(ci, CPS)
                    col = g * GSEGS + si
                    nc.tensor.matmul(
                        pool_acc[:, col:col + 1],
                        lhsT=xn_g[:, ci * 128:(ci + 1) * 128],
                        rhs=alphab[:, ci:ci + 1],
                        start=(g == 0 and ci == 0),
                        stop=(g == GROUPS - 1 and ci == GCH - 1),
                    )

            prev = None
            for g in range(GROUPS):
                xn_g = px_n.tile([128, GW], bf16)
                nc.sync.dma_start(xn_g[:], xn[:, g * GW:(g + 1) * GW])
                xt_g = px_t.tile([128, GW], bf16)
                nc.sync.dma_start(xt_g[:], xt[:, g * GW:(g + 1) * GW])

                # scores for group g
                s_g = pp_s.tile([128, GCH], f32)
                for si in range(GSEGS):
                    base = si * SMAX
                    h_a = pp_h.tile([128, 512], f32, tag="h_a")
                    h_b = pp_h.tile([128, 128], f32, tag="h_b")
                    nc.tensor.matmul(h_a[:], lhsT=w1_sb[:],
                                     rhs=xt_g[:, base:base + 512],
                                     start=True, stop=True)
                    nc.tensor.matmul(h_b[:], lhsT=w1_sb[:],
                                     rhs=xt_g[:, base + 512:base + 640],
                                     start=True, stop=True)
                    tanhT = ptan.tile([128, SMAX], bf16)
                    nc.scalar.activation(tanhT[:, :512], h_a[:], TANH,
                                         bias=b1_sb[:, :1])
                    nc.scalar.activation(tanhT[:, 512:], h_b[:], TANH,
                                         bias=b1_sb[:, :1])
                    for t in range(CPS):
                        k = si * CPS + t
                        nc.tensor.matmul(
                            s_g[:, k:k + 1],
                            lhsT=tanhT[:, t * 128:(t + 1) * 128],
                            rhs=w2_sb[:],
                            start=(k == 0), stop=(k == GCH - 1),
                        )

                # pooling for group g-1 (keeps PE busy while ACT/DVE do softmax g)
                if prev is not None:
                    emit_pool(*prev)

                # segment softmax for group g
                s_sb = pe.tile([128, GCH], f32, tag="s_sb")
                nc.vector.tensor_tensor(out=s_sb[:], in0=s_g[:],
                                        in1=mask_sb[:, g * GCH:(g + 1) * GCH],
                                        op=ADD)
                e_sb = pe.tile([128, GCH], f32, tag="e_sb")
                nc.scalar.activation(e_sb[:], s_sb[:], EXP)
                colsum = pp_c.tile([128, GCH], f32)
                nc.tensor.matmul(colsum[:], lhsT=ones_sb[:], rhs=e_sb[:],
                                 start=True, stop=True)
                sums = pe.tile([128, GSEGS], f32, tag="sums")
                nc.vector.tensor_reduce(
                    out=sums[:],
                    in_=colsum[:].rearrange("p (s t) -> p s t", t=CPS),
                    axis=AXX, op=ADD)
                nc.vector.tensor_scalar_add(sums[:], sums[:], 1e-16)
                recip = pe.tile([128, GSEGS], f32, tag="recip")
                nc.vector.reciprocal(recip[:], sums[:])
                for si in range(GSEGS):
                    cbase = (g * GSEGS + si) * CPS
                    nc.vector.tensor_scalar_mul(
                        alpha_sb[:, cbase:cbase + CPS],
                        e_sb[:, si * CPS:(si + 1) * CPS],
                        recip[:, si:si + 1])
                alphab = pe.tile([128, GCH], bf16, tag="alphab")
                nc.vector.tensor_copy(out=alphab[:],
                                      in_=alpha_sb[:, g * GCH:(g + 1) * GCH])

                prev = (xn_g, alphab, g)

            emit_pool(*prev)

            nc.vector.tensor_copy(out=out_sb[:], in_=pool_acc[:])
            nc.sync.dma_start(outp[:, :], out_sb[:])
            nc.sync.dma_start(alph[:, :], alpha_sb[:])

    nc.finalize()
    return nc


def _prep_inputs(x, W1, b1, W2, b2, batch_idx):
    """Host-side shard/pad/swizzle.  Returns (in_maps, starts, lens)."""
    x = np.ascontiguousarray(x, dtype=np.float32)
    bi = np.ascontiguousarray(batch_idx).astype(np.int64)
    starts = np.searchsorted(bi, np.arange(NSEG + 1)).astype(np.int64)
    lens = np.diff(starts)
    assert lens.max() <= SMAX, f"segment length {lens.max()} > SMAX={SMAX}"

    b2f = float(np.asarray(b2).reshape(-1)[0])
    w1b = np.ascontiguousarray(W1, dtype=np.float32).astype(BF16)
    w2b = np.ascontiguousarray(W2, dtype=np.float32).reshape(128, 1).astype(BF16)
    b1f = np.ascontiguousarray(b1, dtype=np.float32).reshape(128, 1)

    in_maps = []
    col = np.arange(SMAX)
    for c in range(NCORES):
        s0 = c * SPC
        lens_c = lens[s0:s0 + SPC]
        valid = col[None, :] < lens_c[:, None]          # [512, 640]
        xpad = np.zeros((SPC, SMAX, 128), dtype=BF16)
        xpad[valid] = x[starts[s0]:starts[s0 + SPC]].astype(BF16)
        x3 = xpad.reshape(TOT_CH, 128, 128)
        xn_host = np.ascontiguousarray(x3.transpose(1, 0, 2)).reshape(128, RPAD)
        xt_host = np.ascontiguousarray(x3.transpose(2, 0, 1)).reshape(128, RPAD)
        mb = np.where(valid, b2f, np.float32(-1e30)).astype(np.float32)
        mask_host = np.ascontiguousarray(
            mb.reshape(SPC, CPS, 128).transpose(2, 0, 1)).reshape(128, TOT_CH)
        in_maps.append({
            "xn": xn_host, "xt": xt_host, "maskb": mask_host,
            "w1": w1b, "w2": w2b, "b1d": b1f,
        })
    return in_maps, starts, lens


def _install_trace_shim():
    """Optional: register the axon NTFF profile hook so BASS_TRACE=1 works.

    The agent image lacks ``antenv.axon_hooks``; synthesize it and register
    the ctypes hook from the axon boot helper.  Harmless if anything is
    missing — tracing just stays disabled.
    """
    try:
        import sys
        import types
        if "antenv.axon_hooks" in sys.modules:
            return
        mod = types.ModuleType("antenv.axon_hooks")
        _h = [None]
        mod.set_axon_ntff_profile_hook = lambda h: _h.__setitem__(0, h)
        mod.get_axon_ntff_profile_hook = lambda: _h[0]
        sys.modules["antenv.axon_hooks"] = mod
        try:
            import antenv
            antenv.axon_hooks = mod
        except ImportError:
            pass
        sys.path.insert(0, "/root/.axon_site")
        from trn_agent_boot.trn_boot import _ntff_profile_via_ctypes
        hook = _ntff_profile_via_ctypes("/opt/axon/libaxon_pjrt.so")
        if hook is not None:
            mod.set_axon_ntff_profile_hook(hook)
        import concourse.bass_utils as bu
        bu.upload_artifacts = lambda tmpdir: tmpdir  # no bucket in this env
    except Exception:
        pass


def kernel(x, W1, b1, W2, b2, batch_idx, dim_size):
    import os
    if os.environ.get("BASS_TRACE"):
        _install_trace_shim()
    from concourse.bass_utils import run_bass_kernel_spmd

    if "nc" not in _CACHE:
        _CACHE["nc"] = _build_nc()
    nc = _CACHE["nc"]

    in_maps, starts, lens = _prep_inputs(x, W1, b1, W2, b2, batch_idx)
    res = run_bass_kernel_spmd(nc, in_maps, core_ids=list(range(NCORES)))
    _CACHE["last_result"] = res

    out = np.zeros((NSEG, 128), dtype=np.float32)
    alpha = np.zeros((N,), dtype=np.float32)
    col = np.arange(SMAX)
    for c in range(NCORES):
        s0 = c * SPC
        r = res.results[c]
        out[s0:s0 + SPC] = r["outp"].T
        ap = r["alph"].reshape(128, SPC, CPS).transpose(1, 2, 0).reshape(SPC, SMAX)
        valid = col[None, :] < lens[s0:s0 + SPC, None]
        alpha[starts[s0]:starts[s0 + SPC]] = ap[valid]
    return out, alpha[:, None]
